# revision 14
# baseline (speedup 1.0000x reference)
"""GCNBlock (GraphSAGE mean conv + LayerNorm) Trainium2 kernel.

Problem shapes (hardcoded): B=8, N=8192, F_IN=F_OUT=64, 8 NeuronCores.

Math (reference):
    A    = (adj > 0)                      # [N, N], values in {0, 1}
    deg  = A.sum(1)
    agg  = (A @ x[b]) / max(deg, 1)       # per batch b
    out  = relu(x @ W_self + agg @ W_neigh (+ biases))
    out  = LayerNorm(out) * gamma + beta  # over feature dim, eps=1e-5

Restructuring (exact in real arithmetic):
  * (A @ x)/deg @ W_neigh == (A @ (x W_neigh))/deg, so W_neigh folds into
    the streamed activations: y = x @ W_neigh.
  * relu commutes with positive per-row scaling and LayerNorm is invariant
    to scaling of each feature vector, so instead of dividing the
    aggregation by deg we multiply the self path by deg:
        LN(relu(s + (A@y)_i / max(deg_i,1)))
          == LN(relu(max(deg_i,1) * s + (A@y)_i))
    with s = x W_self + b_self + (deg>0)*b_neigh computed (and deg-scaled)
    on the host.  deg==0 rows come out exactly right because (A@y)_i == 0.
  The device then does ONE dense matmul (A @ y) plus an elementwise
  add/relu/LayerNorm epilogue: no transposes, no degree computation, no
  weight matmuls on the PE.

Sharding: 1D row partition of the graph.  Core c owns node rows
[c*1024, (c+1)*1024); adjacency fed pre-transposed (A^T tiles: contraction
dim j on SBUF partitions), y replicated in [j, (b,f)] layout with all 8
batches stacked along the free dim (rhs free dim 512 = one PSUM bank).

Numerics: adjacency AND y in fp8e4m3 (adjacency 0/1 exact; y quantization
contributes ~2.5% error to the neighbor term, which is only ~1.5% of the
output magnitude -> ~0.04% output error).  Both operands fp8 enables the
PE DoubleRow perf mode: each matmul consumes TWO 128-row j-tiles,
doubling effective throughput vs fp16.  Accumulation fp32 (PSUM); self
path fp16; LN stats fp32; output fp16 (upcast on host).  gamma/beta
applied on the host (exact affine; ones/zeros here).

Per-core schedule:
  ramp:   first G=4 row-tiles accumulate j-interleaved so the replicated y
          stream (4 MB) amortizes 4x while adjacency streams at ~150 GB/s.
  steady: remaining row-tiles sequential; y fully SBUF-resident by then.
  DMA:    both HWDGE rings (SP + ACT) carry an interleaved share of every
          stream (at/y/ss/out) so neither ring starves the other at the
          packet-arbitration level; ss rides early between y pieces; the
          sequential adjacency is enqueued before the ramp epilogues'
          output DMAs to avoid head-of-line blocking.
  epilogue per row-tile (engines split DVE/ACT):
          DVE add (PSUM agg + ss) -> ACT relu -> ONE grouped bn_stats
          [128,8,64] -> even/odd moment merge (tiny [128,8] ops) ->
          ACT sqrt -> DVE reciprocal -> normalize, 4 segments on DVE
          (tensor_scalar) + 4 on ACT (Identity with scale/bias APs).

HW exec: ~70 us (PE ~225 ns/DoubleRow pair incl. weight-load overhead).
"""

import numpy as np
import ml_dtypes

import concourse.bass as bass
import concourse.mybir as mybir
from concourse.tile import TileContext
from concourse.masks import make_identity
from concourse.bass_utils import run_bass_kernel_spmd

B, N, F = 8, 8192, 64
N_CORES = 8
R = N // N_CORES          # rows (nodes) per core = 1024
IT = R // 128             # row-tiles per core = 8
JT = N // 128             # contraction tiles = 64
NP = JT // 2              # DoubleRow j-tile pairs = 32
BF = B * F                # stacked batch*feature free dim = 512
G = 4                     # row-tiles interleaved during the ramp
AQ = 16                   # j-tiles per adjacency load piece (steady state)
LN_EPS = 1e-5

_F16 = mybir.dt.float16
_F32 = mybir.dt.float32
_F8 = mybir.dt.float8e4
_DR = mybir.MatmulPerfMode.DoubleRow
_OP = mybir.AluOpType
_AF = mybir.ActivationFunctionType


def _build_bass() -> bass.Bass:
    nc = bass.Bass()

    # Host-side layouts (see _prep_inputs):
    #   at : [IT, 128 p, JT, 128 i] fp8, p = j-within-tile (A^T tiles)
    #   y  : [128 p, JT, BF]        fp8, y[p, jt, b*64+f] = (x@Wn)[b, jt*128+p, f]
    #   ss : [IT, 128 p, BF]        fp16, deg-scaled self path
    at = nc.dram_tensor("at", (IT, 128, JT, 128), _F8, kind="ExternalInput")
    y = nc.dram_tensor("y", (128, JT, BF), _F8, kind="ExternalInput")
    ss = nc.dram_tensor("ss", (IT, 128, BF), _F16, kind="ExternalInput")
    out = nc.dram_tensor("out", (IT, 128, BF), _F16, kind="ExternalOutput")

    with TileContext(nc) as tc:
        with (
            tc.tile_pool(name="consts", bufs=1) as consts,
            tc.tile_pool(name="yp", bufs=18) as yp,
            tc.tile_pool(name="atp", bufs=44) as atp,
            tc.tile_pool(name="ssp", bufs=IT) as ssp,
            tc.tile_pool(name="rp", bufs=3) as rp,
            tc.tile_pool(name="lnp", bufs=3) as lnp,
            tc.tile_pool(name="outp", bufs=3) as outp,
            tc.tile_pool(name="ps_agg", bufs=6, space="PSUM") as ps_agg,
        ):
            qs = [nc.sync, nc.scalar]   # the two HWDGE rings
            qi = 0

            def q():
                nonlocal qi
                qi += 1
                return qs[qi % 2]

            # ---- input streams, emitted in CONSUMPTION order and alternated
            # across both HWDGE rings DMA-by-DMA, so each ring carries half of
            # every stream and queue order matches the PE's needs.  ss and out
            # ride the gpsimd SWDGE queue, keeping the rings pure input.
            #   y pieces:  jt starts 0,2,4,8,..,60
            #   at ramp:   blocks [4,4,8,16,16,16] x tiles 0..G-1
            #   at prefetch for tiles 4,5 interleaved into the ramp tail
            #   (fresh pool slots: 24 ramp + 8 prefetch = 32 bufs, no reuse).
            y_sizes = {0: 2, 2: 2}
            y_sizes.update({4 + 4 * k: 4 for k in range(15)})
            at_blocks = {0: 2, 2: 2, 4: 4, 8: 8, 16: 16, 32: 16, 48: 16}
            y_tiles = []
            luts = {g: [] for g in range(IT)}
            yk = 0
            for jt in range(JT):
                if jt in y_sizes:
                    sz = y_sizes[jt]
                    y_sb = yp.tile([128, sz, BF], _F8, name=f"y{yk}", tag="y",
                                   padded_shape=[128, 4, BF])
                    q().dma_start(out=y_sb, in_=y[:, jt:jt + sz, :])
                    y_tiles.extend((y_sb, l) for l in range(sz))
                    yk += 1
                if jt in at_blocks:
                    sz = at_blocks[jt]
                    for g in range(G):
                        at_q = atp.tile([128, sz, 128], _F8, name="at_q",
                                        tag="at_q", padded_shape=[128, AQ, 128])
                        q().dma_start(out=at_q, in_=at[g, :, jt:jt + sz, :])
                        luts[g].extend((at_q, l) for l in range(sz))
            # Prefetch the first sequential tiles' adjacency at the stream
            # tail so it never delays the ramp's own bytes.
            for it in (4, 5):
                for p in range(JT // AQ):
                    at_q = atp.tile([128, AQ, 128], _F8, name="at_q",
                                    tag="at_q", padded_shape=[128, AQ, 128])
                    q().dma_start(out=at_q, in_=at[it, :, p * AQ:(p + 1) * AQ, :])
                    luts[it].extend((at_q, l) for l in range(AQ))

            # ss on the SWDGE queue (small, late-needed).
            ss_tiles = []
            for it in range(IT):
                s_sb = ssp.tile([128, BF], _F16, name=f"ss{it}", tag="ss")
                nc.gpsimd.dma_start(out=s_sb, in_=ss[it])
                ss_tiles.append(s_sb)

            eps = consts.tile([128, 1], _F32)
            nc.vector.memset(eps, LN_EPS / 65536.0)
            ident = consts.tile([128, 128], _F16)
            make_identity(nc, ident)

            # ---- ramp matmuls: pair-major across the first G row-tiles.
            aggs = {g: ps_agg.tile([128, BF], _F32, name=f"agg{g}", tag="agg")
                    for g in range(G)}
            for m in range(NP):
                yt, yl = y_tiles[2 * m]
                for g in range(G):
                    att, al = luts[g][2 * m]
                    nc.tensor.matmul(
                        aggs[g], lhsT=att[:, al:al + 2, :], rhs=yt[:, yl:yl + 2, :],
                        start=(m == 0), stop=False, perf_mode=_DR,
                    )
            # Fold the deg-scaled self path into PSUM on the PE (identity
            # stationary): frees the DVE of the add and lets the relu read
            # PSUM directly; closes each ramp tile's accumulation group.
            for g in range(G):
                nc.tensor.matmul(
                    aggs[g], lhsT=ident, rhs=ss_tiles[g], start=False, stop=True,
                )

            # ---- adjacency for the last row-tiles (6,7): fresh slots are
            # gone, so allocate AFTER the ramp matmuls are emitted (pool reuse
            # needs the previous readers on record to wait on).
            for it in range(G + 2, IT):
                for p in range(JT // AQ):
                    at_q = atp.tile([128, AQ, 128], _F8, name="at_q", tag="at_q",
                                    padded_shape=[128, AQ, 128])
                    q().dma_start(out=at_q, in_=at[it, :, p * AQ:(p + 1) * AQ, :])
                    luts[it].extend((at_q, l) for l in range(AQ))

            def backend(it, agg, ns=8, coff=0):
                # PSUM already holds deg-scaled pre-relu values (self path
                # folded in via the identity matmul).  All stats work on
                # 1/256-scaled values (relu applies the scale) so squares fit
                # fp16; LayerNorm is invariant to the scaling (eps adjusted).
                # Covers `ns` 64-col segments starting at column `coff`.
                # r[:, 0] = relu(agg/256), r[:, 1] = its square, per 64-seg.
                r = rp.tile([128, 2, ns, 64], _F16, tag="r",
                            padded_shape=[128, 2, 8, 64])
                nc.scalar.activation(
                    out=r[:, 0], in_=agg, func=_AF.Relu, scale=1.0 / 256.0)
                # S first so the DVE overlaps ACT's square pass.
                sq = lnp.tile([128, 2, ns], _F32, tag="sq",
                              padded_shape=[128, 2, 8])
                S, Q = sq[:, 0, :], sq[:, 1, :]
                nc.vector.tensor_reduce(
                    out=sq[:, 0, :], in_=r[:, 0], axis=mybir.AxisListType.X,
                    op=_OP.add)
                s2 = lnp.tile([128, ns], _F32, tag="s2", padded_shape=[128, 8])
                nc.vector.scalar_tensor_tensor(
                    out=s2, in0=S, scalar=1.0, in1=S, op0=_OP.mult, op1=_OP.mult)
                mean = lnp.tile([128, ns], _F32, tag="mean",
                                padded_shape=[128, 8])      # S/64
                nc.vector.tensor_scalar_mul(out=mean, in0=S, scalar1=1.0 / 64.0)
                nc.scalar.activation(out=r[:, 1], in_=r[:, 0], func=_AF.Square)
                nc.vector.tensor_reduce(
                    out=sq[:, 1, :], in_=r[:, 1], axis=mybir.AxisListType.X,
                    op=_OP.add)
                v = lnp.tile([128, ns], _F32, tag="v", padded_shape=[128, 8])
                nc.vector.scalar_tensor_tensor(
                    out=v, in0=Q, scalar=64.0, in1=s2,
                    op0=_OP.mult, op1=_OP.subtract)         # 64Q - S^2 = 4096*var
                # std = sqrt(v/4096 + eps'); rstd = 1/std
                std = lnp.tile([128, ns], _F32, tag="std", padded_shape=[128, 8])
                nc.scalar.activation(
                    out=std, in_=v, func=_AF.Sqrt, bias=eps, scale=1.0 / 4096.0)
                rstd = lnp.tile([128, ns], _F32, tag="rstd", padded_shape=[128, 8])
                nc.vector.reciprocal(out=rstd, in_=std)
                nmr = lnp.tile([128, ns], _F32, tag="nmr",
                               padded_shape=[128, 8])       # -mean*rstd
                nc.vector.scalar_tensor_tensor(
                    out=nmr, in0=mean, scalar=-1.0, in1=rstd,
                    op0=_OP.mult, op1=_OP.mult)
                o = outp.tile([128, ns * 64], _F16, tag="o",
                              padded_shape=[128, BF])
                for seg in range(ns):
                    oseg = o[:, seg * 64:(seg + 1) * 64]
                    if seg < (5 * ns) // 8:
                        nc.vector.tensor_scalar(
                            out=oseg, in0=r[:, 0, seg, :],
                            scalar1=mean[:, seg:seg + 1],
                            scalar2=rstd[:, seg:seg + 1],
                            op0=_OP.subtract, op1=_OP.mult,
                        )
                    else:
                        nc.scalar.activation(
                            out=oseg, in_=r[:, 0, seg, :], func=_AF.Identity,
                            scale=rstd[:, seg:seg + 1], bias=nmr[:, seg:seg + 1],
                        )
                q().dma_start(out=out[it, :, coff:coff + ns * 64], in_=o)

            for g in range(G):
                backend(g, aggs[g])

            # ---- steady state: remaining row-tiles sequential, y resident.
            for it in range(G, IT - 1):
                agg = ps_agg.tile([128, BF], _F32, tag="agg")
                for m in range(NP):
                    att, al = luts[it][2 * m]
                    yt, yl = y_tiles[2 * m]
                    nc.tensor.matmul(
                        agg, lhsT=att[:, al:al + 2, :], rhs=yt[:, yl:yl + 2, :],
                        start=(m == 0), stop=False, perf_mode=_DR,
                    )
                nc.tensor.matmul(
                    agg, lhsT=ident, rhs=ss_tiles[it], start=False, stop=True,
                )
                backend(it, agg)

            # ---- last row-tile: split into column halves so the first
            # half's epilogue overlaps the second half's matmuls, shrinking
            # the post-matmul tail.
            it = IT - 1
            for h in range(2):
                cols = slice(h * 256, (h + 1) * 256)
                aggh = ps_agg.tile([128, 256], _F32, tag="agg",
                                   padded_shape=[128, BF])
                for m in range(NP):
                    att, al = luts[it][2 * m]
                    yt, yl = y_tiles[2 * m]
                    nc.tensor.matmul(
                        aggh, lhsT=att[:, al:al + 2, :],
                        rhs=yt[:, yl:yl + 2, cols],
                        start=(m == 0), stop=False, perf_mode=_DR,
                    )
                nc.tensor.matmul(
                    aggh, lhsT=ident, rhs=ss_tiles[it][:, cols],
                    start=False, stop=True,
                )
                backend(it, aggh, ns=4, coff=h * 256)

    return nc


def _split_multi_waits(nc: bass.Bass) -> None:
    """This walrus build rejects any instruction carrying more than one sync
    wait ("Too many sync wait commands").  Tile's wait emission is per-proc
    minimal but not transitively so, and happily puts several waits on one
    instruction.  Equivalent fix: peel all but the last wait onto same-engine
    NOPs issued immediately before it (engine queues are strict FIFO, so the
    sequencer blocks on each in turn)."""
    from concourse.mybir import SyncInfo

    nid = 0
    for blk in nc.m.functions[0].blocks:
        out = []
        for inst in blk.instructions:
            si = getattr(inst, "sync_info", None)
            if si is not None and len(si.on_wait) > 1:
                waits = list(si.on_wait)
                for w in waits[:-1]:
                    nop = mybir.InstNoOp(name=f"wait_nop_{nid}")
                    nid += 1
                    nop.engine = inst.engine
                    nop.sync_info = SyncInfo(on_wait=[w], on_update=[])
                    out.append(nop)
                inst.sync_info = SyncInfo(
                    on_wait=[waits[-1]],
                    on_update=list(si.on_update),
                )
            out.append(inst)
        blk.instructions[:] = out


_NC_CACHE = None


def _get_nc() -> bass.Bass:
    global _NC_CACHE
    if _NC_CACHE is None:
        _NC_CACHE = _build_bass()
        _split_multi_waits(_NC_CACHE)
    return _NC_CACHE


def _prep_inputs(x, adj_matrix, W_self, b_self, W_neigh, b_neigh):
    """Host-side shard + layout prep and weight folding (see module doc)."""
    x = np.asarray(x, dtype=np.float32)
    A = np.asarray(adj_matrix) > 0                      # [N, N] bool
    deg = A.sum(axis=1).astype(np.float32)              # [N]
    degc = np.maximum(deg, 1.0)

    wn = np.asarray(W_neigh, np.float32)
    ws = np.asarray(W_self, np.float32)
    bs = np.asarray(b_self, np.float32)
    bn = np.asarray(b_neigh, np.float32)

    # y[p, jt, b*64+f] = (x @ W_neigh)[b, jt*128+p, f]; replicated to cores.
    yv = (x.reshape(-1, F) @ wn).reshape(B, N, F)
    y2 = yv.transpose(1, 0, 2).reshape(N, BF)           # [n, bf]
    y_host = np.ascontiguousarray(
        y2.reshape(JT, 128, BF).transpose(1, 0, 2)
    ).astype(ml_dtypes.float8_e4m3fn)                   # [128 p, JT, BF]

    # Deg-scaled self path: max(deg,1) * (x W_self + b_self + (deg>0) b_neigh).
    sv = (x.reshape(-1, F) @ ws).reshape(B, N, F) + bs[None, None, :]
    sv = sv + (deg > 0).astype(np.float32)[None, :, None] * bn[None, None, :]
    sv = sv * degc[None, :, None]
    ss2 = sv.transpose(1, 0, 2).reshape(N, BF)          # [n, bf]

    in_maps = []
    for c in range(N_CORES):
        rows = slice(c * R, (c + 1) * R)
        # at[it, p, jt, i] = A[c*1024 + it*128 + i, jt*128 + p]
        blk = A[rows].reshape(IT, 128, JT, 128)         # [it, i, jt, p]
        at_c = np.ascontiguousarray(
            blk.transpose(0, 3, 2, 1)
        ).astype(ml_dtypes.float8_e4m3fn)               # [it, p, jt, i], exact 0/1
        ss_c = np.ascontiguousarray(
            ss2[rows].reshape(IT, 128, BF)
        ).astype(np.float16)
        in_maps.append({"at": at_c, "y": y_host, "ss": ss_c})
    return in_maps


def _run(inputs: dict, trace: bool = False):
    in_maps = _prep_inputs(
        inputs["x"], inputs["adj_matrix"], inputs["W_self"], inputs["b_self"],
        inputs["W_neigh"], inputs["b_neigh"],
    )
    nc = _get_nc()
    res = run_bass_kernel_spmd(nc, in_maps, core_ids=list(range(N_CORES)), trace=trace)

    out_full = np.empty((B, N, F), dtype=np.float32)
    for c in range(N_CORES):
        oc = np.asarray(res.results[c]["out"], dtype=np.float32)  # [IT, 128, BF]
        out_full[:, c * R:(c + 1) * R, :] = (
            oc.reshape(R, B, F).transpose(1, 0, 2)
        )

    # Exact host-side affine epilogue (gamma/beta are data, not compile-time).
    gamma = np.asarray(inputs["ln_gamma"], np.float32)
    beta = np.asarray(inputs["ln_beta"], np.float32)
    if not (np.all(gamma == 1.0) and np.all(beta == 0.0)):
        out_full = out_full * gamma + beta
    return out_full, res


def kernel(**inputs) -> np.ndarray:
    out, _ = _run(inputs, trace=False)
    return out


# revision 15
# speedup vs baseline: 1.0451x; 1.0451x over previous
"""GCNBlock (GraphSAGE mean conv + LayerNorm) Trainium2 kernel.

Problem shapes (hardcoded): B=8, N=8192, F_IN=F_OUT=64, 8 NeuronCores.

Math (reference):
    A    = (adj > 0)                      # [N, N], values in {0, 1}
    deg  = A.sum(1)
    agg  = (A @ x[b]) / max(deg, 1)       # per batch b
    out  = relu(x @ W_self + agg @ W_neigh (+ biases))
    out  = LayerNorm(out) * gamma + beta  # over feature dim, eps=1e-5

Restructuring (exact in real arithmetic):
  * (A @ x)/deg @ W_neigh == (A @ (x W_neigh))/deg, so W_neigh folds into
    the streamed activations: y = x @ W_neigh.
  * relu commutes with positive per-row scaling and LayerNorm is invariant
    to scaling of each feature vector, so instead of dividing the
    aggregation by deg we multiply the self path by deg:
        LN(relu(s + (A@y)_i / max(deg_i,1)))
          == LN(relu(max(deg_i,1) * s + (A@y)_i))
    with s = x W_self + b_self + (deg>0)*b_neigh computed (and deg-scaled)
    on the host.  deg==0 rows come out exactly right because (A@y)_i == 0.
  The device then does ONE dense matmul (A @ y) plus an elementwise
  add/relu/LayerNorm epilogue: no transposes, no degree computation, no
  weight matmuls on the PE.

Sharding: 1D row partition of the graph.  Core c owns node rows
[c*1024, (c+1)*1024); adjacency fed pre-transposed (A^T tiles: contraction
dim j on SBUF partitions), y replicated in [j, (b,f)] layout with all 8
batches stacked along the free dim (rhs free dim 512 = one PSUM bank).

Numerics: adjacency AND y in fp8e4m3 (adjacency 0/1 exact; y quantization
contributes ~2.5% error to the neighbor term, which is only ~1.5% of the
output magnitude -> ~0.04% output error).  Both operands fp8 enables the
PE DoubleRow perf mode: each matmul consumes TWO 128-row j-tiles,
doubling effective throughput vs fp16.  Accumulation fp32 (PSUM); self
path fp16; LN stats fp32; output fp16 (upcast on host).  gamma/beta
applied on the host (exact affine; ones/zeros here).

Per-core schedule:
  ramp:   first G=4 row-tiles accumulate j-interleaved so the replicated y
          stream (4 MB) amortizes 4x while adjacency streams at ~150 GB/s.
  steady: remaining row-tiles sequential; y fully SBUF-resident by then.
  DMA:    both HWDGE rings (SP + ACT) carry an interleaved share of every
          stream (at/y/ss/out) so neither ring starves the other at the
          packet-arbitration level; ss rides early between y pieces; the
          sequential adjacency is enqueued before the ramp epilogues'
          output DMAs to avoid head-of-line blocking.
  epilogue per row-tile (engines split DVE/ACT):
          DVE add (PSUM agg + ss) -> ACT relu -> ONE grouped bn_stats
          [128,8,64] -> even/odd moment merge (tiny [128,8] ops) ->
          ACT sqrt -> DVE reciprocal -> normalize, 4 segments on DVE
          (tensor_scalar) + 4 on ACT (Identity with scale/bias APs).

HW exec: ~70 us (PE ~225 ns/DoubleRow pair incl. weight-load overhead).
"""

import numpy as np
import ml_dtypes

import concourse.bass as bass
import concourse.mybir as mybir
from concourse.tile import TileContext
from concourse.masks import make_identity
from concourse.bass_utils import run_bass_kernel_spmd

B, N, F = 8, 8192, 64
N_CORES = 8
R = N // N_CORES          # rows (nodes) per core = 1024
IT = R // 128             # row-tiles per core = 8
JT = N // 128             # contraction tiles = 64
NP = JT // 2              # DoubleRow j-tile pairs = 32
BF = B * F                # stacked batch*feature free dim = 512
G = 4                     # row-tiles interleaved during the ramp
AQ = 16                   # j-tiles per adjacency load piece (steady state)
LN_EPS = 1e-5

_F16 = mybir.dt.float16
_F32 = mybir.dt.float32
_F8 = mybir.dt.float8e4
_DR = mybir.MatmulPerfMode.DoubleRow
_OP = mybir.AluOpType
_AF = mybir.ActivationFunctionType


def _build_bass() -> bass.Bass:
    nc = bass.Bass()

    # Host-side layouts (see _prep_inputs):
    #   at : [IT, 128 p, JT, 128 i] fp8, p = j-within-tile (A^T tiles)
    #   y  : [128 p, JT, BF]        fp8, y[p, jt, b*64+f] = (x@Wn)[b, jt*128+p, f]
    #   ss : [IT, 128 p, BF]        fp16, deg-scaled self path
    at = nc.dram_tensor("at", (IT, 128, JT, 128), _F8, kind="ExternalInput")
    y = nc.dram_tensor("y", (128, JT, BF), _F8, kind="ExternalInput")
    ss = nc.dram_tensor("ss", (IT, 128, BF), _F16, kind="ExternalInput")
    out = nc.dram_tensor("out", (IT, 128, BF), _F16, kind="ExternalOutput")

    with TileContext(nc) as tc:
        with (
            tc.tile_pool(name="consts", bufs=1) as consts,
            tc.tile_pool(name="yp", bufs=18) as yp,
            tc.tile_pool(name="atp", bufs=44) as atp,
            tc.tile_pool(name="ssp", bufs=IT) as ssp,
            tc.tile_pool(name="rp", bufs=3) as rp,
            tc.tile_pool(name="lnp", bufs=3) as lnp,
            tc.tile_pool(name="outp", bufs=3) as outp,
            tc.tile_pool(name="ps_agg", bufs=6, space="PSUM") as ps_agg,
        ):
            qs = [nc.sync, nc.scalar]   # the two HWDGE rings
            qi = 0

            def q():
                nonlocal qi
                qi += 1
                return qs[qi % 2]

            # ---- input streams, emitted in CONSUMPTION order and alternated
            # across both HWDGE rings DMA-by-DMA, so each ring carries half of
            # every stream and queue order matches the PE's needs.  ss and out
            # ride the gpsimd SWDGE queue, keeping the rings pure input.
            #   y pieces:  jt starts 0,2,4,8,..,60
            #   at ramp:   blocks [4,4,8,16,16,16] x tiles 0..G-1
            #   at prefetch for tiles 4,5 interleaved into the ramp tail
            #   (fresh pool slots: 24 ramp + 8 prefetch = 32 bufs, no reuse).
            y_sizes = {0: 2, 2: 2}
            y_sizes.update({4 + 4 * k: 4 for k in range(15)})
            at_blocks = {0: 8, 8: 8, 16: 16, 32: 16, 48: 16}
            y_tiles = []
            luts = {g: [] for g in range(IT)}
            yk = 0
            for jt in range(JT):
                if jt in y_sizes:
                    sz = y_sizes[jt]
                    y_sb = yp.tile([128, sz, BF], _F8, name=f"y{yk}", tag="y",
                                   padded_shape=[128, 4, BF])
                    q().dma_start(out=y_sb, in_=y[:, jt:jt + sz, :])
                    y_tiles.extend((y_sb, l) for l in range(sz))
                    yk += 1
                if jt in at_blocks:
                    sz = at_blocks[jt]
                    for g in range(G):
                        at_q = atp.tile([128, sz, 128], _F8, name="at_q",
                                        tag="at_q", padded_shape=[128, AQ, 128])
                        q().dma_start(out=at_q, in_=at[g, :, jt:jt + sz, :])
                        luts[g].extend((at_q, l) for l in range(sz))
            # Prefetch the first sequential tiles' adjacency at the stream
            # tail so it never delays the ramp's own bytes.
            for it in (4, 5):
                for p in range(JT // AQ):
                    at_q = atp.tile([128, AQ, 128], _F8, name="at_q",
                                    tag="at_q", padded_shape=[128, AQ, 128])
                    q().dma_start(out=at_q, in_=at[it, :, p * AQ:(p + 1) * AQ, :])
                    luts[it].extend((at_q, l) for l in range(AQ))

            # ss on the SWDGE queue (small, late-needed).
            ss_tiles = []
            for it in range(IT):
                s_sb = ssp.tile([128, BF], _F16, name=f"ss{it}", tag="ss")
                nc.gpsimd.dma_start(out=s_sb, in_=ss[it])
                ss_tiles.append(s_sb)

            eps = consts.tile([128, 1], _F32)
            nc.vector.memset(eps, LN_EPS / 65536.0)
            ident = consts.tile([128, 128], _F16)
            make_identity(nc, ident)

            # ---- ramp matmuls: pair-major across the first G row-tiles.
            aggs = {g: ps_agg.tile([128, BF], _F32, name=f"agg{g}", tag="agg")
                    for g in range(G)}
            for m in range(NP):
                yt, yl = y_tiles[2 * m]
                for g in range(G):
                    att, al = luts[g][2 * m]
                    nc.tensor.matmul(
                        aggs[g], lhsT=att[:, al:al + 2, :], rhs=yt[:, yl:yl + 2, :],
                        start=(m == 0), stop=False, perf_mode=_DR,
                    )
            # Fold the deg-scaled self path into PSUM on the PE (identity
            # stationary): frees the DVE of the add and lets the relu read
            # PSUM directly; closes each ramp tile's accumulation group.
            for g in range(G):
                nc.tensor.matmul(
                    aggs[g], lhsT=ident, rhs=ss_tiles[g], start=False, stop=True,
                )

            # ---- adjacency for the last row-tiles (6,7): fresh slots are
            # gone, so allocate AFTER the ramp matmuls are emitted (pool reuse
            # needs the previous readers on record to wait on).
            for it in range(G + 2, IT):
                for p in range(JT // AQ):
                    at_q = atp.tile([128, AQ, 128], _F8, name="at_q", tag="at_q",
                                    padded_shape=[128, AQ, 128])
                    q().dma_start(out=at_q, in_=at[it, :, p * AQ:(p + 1) * AQ, :])
                    luts[it].extend((at_q, l) for l in range(AQ))

            def backend(it, agg, ns=8, coff=0):
                # PSUM already holds deg-scaled pre-relu values (self path
                # folded in via the identity matmul).  All stats work on
                # 1/256-scaled values (relu applies the scale) so squares fit
                # fp16; LayerNorm is invariant to the scaling (eps adjusted).
                # Covers `ns` 64-col segments starting at column `coff`.
                # r[:, 0] = relu(agg/256), r[:, 1] = its square, per 64-seg.
                r = rp.tile([128, 2, ns, 64], _F16, tag="r",
                            padded_shape=[128, 2, 8, 64])
                nc.scalar.activation(
                    out=r[:, 0], in_=agg, func=_AF.Relu, scale=1.0 / 256.0)
                # S first so the DVE overlaps ACT's square pass.
                sq = lnp.tile([128, 2, ns], _F32, tag="sq",
                              padded_shape=[128, 2, 8])
                S, Q = sq[:, 0, :], sq[:, 1, :]
                nc.vector.tensor_reduce(
                    out=sq[:, 0, :], in_=r[:, 0], axis=mybir.AxisListType.X,
                    op=_OP.add)
                s2 = lnp.tile([128, ns], _F32, tag="s2", padded_shape=[128, 8])
                nc.vector.scalar_tensor_tensor(
                    out=s2, in0=S, scalar=1.0, in1=S, op0=_OP.mult, op1=_OP.mult)
                mean = lnp.tile([128, ns], _F32, tag="mean",
                                padded_shape=[128, 8])      # S/64
                nc.vector.tensor_scalar_mul(out=mean, in0=S, scalar1=1.0 / 64.0)
                nc.scalar.activation(out=r[:, 1], in_=r[:, 0], func=_AF.Square)
                nc.vector.tensor_reduce(
                    out=sq[:, 1, :], in_=r[:, 1], axis=mybir.AxisListType.X,
                    op=_OP.add)
                v = lnp.tile([128, ns], _F32, tag="v", padded_shape=[128, 8])
                nc.vector.scalar_tensor_tensor(
                    out=v, in0=Q, scalar=64.0, in1=s2,
                    op0=_OP.mult, op1=_OP.subtract)         # 64Q - S^2 = 4096*var
                # std = sqrt(v/4096 + eps'); rstd = 1/std
                std = lnp.tile([128, ns], _F32, tag="std", padded_shape=[128, 8])
                nc.scalar.activation(
                    out=std, in_=v, func=_AF.Sqrt, bias=eps, scale=1.0 / 4096.0)
                rstd = lnp.tile([128, ns], _F32, tag="rstd", padded_shape=[128, 8])
                nc.vector.reciprocal(out=rstd, in_=std)
                nmr = lnp.tile([128, ns], _F32, tag="nmr",
                               padded_shape=[128, 8])       # -mean*rstd
                nc.vector.scalar_tensor_tensor(
                    out=nmr, in0=mean, scalar=-1.0, in1=rstd,
                    op0=_OP.mult, op1=_OP.mult)
                o = outp.tile([128, ns * 64], _F16, tag="o",
                              padded_shape=[128, BF])
                for seg in range(ns):
                    oseg = o[:, seg * 64:(seg + 1) * 64]
                    if seg < (5 * ns) // 8:
                        nc.vector.tensor_scalar(
                            out=oseg, in0=r[:, 0, seg, :],
                            scalar1=mean[:, seg:seg + 1],
                            scalar2=rstd[:, seg:seg + 1],
                            op0=_OP.subtract, op1=_OP.mult,
                        )
                    else:
                        nc.scalar.activation(
                            out=oseg, in_=r[:, 0, seg, :], func=_AF.Identity,
                            scale=rstd[:, seg:seg + 1], bias=nmr[:, seg:seg + 1],
                        )
                q().dma_start(out=out[it, :, coff:coff + ns * 64], in_=o)

            for g in range(G):
                backend(g, aggs[g])

            # ---- steady state: remaining row-tiles sequential, y resident.
            for it in range(G, IT - 1):
                agg = ps_agg.tile([128, BF], _F32, tag="agg")
                for m in range(NP):
                    att, al = luts[it][2 * m]
                    yt, yl = y_tiles[2 * m]
                    nc.tensor.matmul(
                        agg, lhsT=att[:, al:al + 2, :], rhs=yt[:, yl:yl + 2, :],
                        start=(m == 0), stop=False, perf_mode=_DR,
                    )
                nc.tensor.matmul(
                    agg, lhsT=ident, rhs=ss_tiles[it], start=False, stop=True,
                )
                backend(it, agg)

            # ---- last row-tile: split into column halves so the first
            # half's epilogue overlaps the second half's matmuls, shrinking
            # the post-matmul tail.
            it = IT - 1
            for h in range(2):
                cols = slice(h * 256, (h + 1) * 256)
                aggh = ps_agg.tile([128, 256], _F32, tag="agg",
                                   padded_shape=[128, BF])
                for m in range(NP):
                    att, al = luts[it][2 * m]
                    yt, yl = y_tiles[2 * m]
                    nc.tensor.matmul(
                        aggh, lhsT=att[:, al:al + 2, :],
                        rhs=yt[:, yl:yl + 2, cols],
                        start=(m == 0), stop=False, perf_mode=_DR,
                    )
                nc.tensor.matmul(
                    aggh, lhsT=ident, rhs=ss_tiles[it][:, cols],
                    start=False, stop=True,
                )
                backend(it, aggh, ns=4, coff=h * 256)

    return nc


def _split_multi_waits(nc: bass.Bass) -> None:
    """This walrus build rejects any instruction carrying more than one sync
    wait ("Too many sync wait commands").  Tile's wait emission is per-proc
    minimal but not transitively so, and happily puts several waits on one
    instruction.  Equivalent fix: peel all but the last wait onto same-engine
    NOPs issued immediately before it (engine queues are strict FIFO, so the
    sequencer blocks on each in turn)."""
    from concourse.mybir import SyncInfo

    nid = 0
    for blk in nc.m.functions[0].blocks:
        out = []
        for inst in blk.instructions:
            si = getattr(inst, "sync_info", None)
            if si is not None and len(si.on_wait) > 1:
                waits = list(si.on_wait)
                for w in waits[:-1]:
                    nop = mybir.InstNoOp(name=f"wait_nop_{nid}")
                    nid += 1
                    nop.engine = inst.engine
                    nop.sync_info = SyncInfo(on_wait=[w], on_update=[])
                    out.append(nop)
                inst.sync_info = SyncInfo(
                    on_wait=[waits[-1]],
                    on_update=list(si.on_update),
                )
            out.append(inst)
        blk.instructions[:] = out


_NC_CACHE = None


def _get_nc() -> bass.Bass:
    global _NC_CACHE
    if _NC_CACHE is None:
        _NC_CACHE = _build_bass()
        _split_multi_waits(_NC_CACHE)
    return _NC_CACHE


def _prep_inputs(x, adj_matrix, W_self, b_self, W_neigh, b_neigh):
    """Host-side shard + layout prep and weight folding (see module doc)."""
    x = np.asarray(x, dtype=np.float32)
    A = np.asarray(adj_matrix) > 0                      # [N, N] bool
    deg = A.sum(axis=1).astype(np.float32)              # [N]
    degc = np.maximum(deg, 1.0)

    wn = np.asarray(W_neigh, np.float32)
    ws = np.asarray(W_self, np.float32)
    bs = np.asarray(b_self, np.float32)
    bn = np.asarray(b_neigh, np.float32)

    # y[p, jt, b*64+f] = (x @ W_neigh)[b, jt*128+p, f]; replicated to cores.
    yv = (x.reshape(-1, F) @ wn).reshape(B, N, F)
    y2 = yv.transpose(1, 0, 2).reshape(N, BF)           # [n, bf]
    y_host = np.ascontiguousarray(
        y2.reshape(JT, 128, BF).transpose(1, 0, 2)
    ).astype(ml_dtypes.float8_e4m3fn)                   # [128 p, JT, BF]

    # Deg-scaled self path: max(deg,1) * (x W_self + b_self + (deg>0) b_neigh).
    sv = (x.reshape(-1, F) @ ws).reshape(B, N, F) + bs[None, None, :]
    sv = sv + (deg > 0).astype(np.float32)[None, :, None] * bn[None, None, :]
    sv = sv * degc[None, :, None]
    ss2 = sv.transpose(1, 0, 2).reshape(N, BF)          # [n, bf]

    in_maps = []
    for c in range(N_CORES):
        rows = slice(c * R, (c + 1) * R)
        # at[it, p, jt, i] = A[c*1024 + it*128 + i, jt*128 + p]
        blk = A[rows].reshape(IT, 128, JT, 128)         # [it, i, jt, p]
        at_c = np.ascontiguousarray(
            blk.transpose(0, 3, 2, 1)
        ).astype(ml_dtypes.float8_e4m3fn)               # [it, p, jt, i], exact 0/1
        ss_c = np.ascontiguousarray(
            ss2[rows].reshape(IT, 128, BF)
        ).astype(np.float16)
        in_maps.append({"at": at_c, "y": y_host, "ss": ss_c})
    return in_maps


def _run(inputs: dict, trace: bool = False):
    in_maps = _prep_inputs(
        inputs["x"], inputs["adj_matrix"], inputs["W_self"], inputs["b_self"],
        inputs["W_neigh"], inputs["b_neigh"],
    )
    nc = _get_nc()
    res = run_bass_kernel_spmd(nc, in_maps, core_ids=list(range(N_CORES)), trace=trace)

    out_full = np.empty((B, N, F), dtype=np.float32)
    for c in range(N_CORES):
        oc = np.asarray(res.results[c]["out"], dtype=np.float32)  # [IT, 128, BF]
        out_full[:, c * R:(c + 1) * R, :] = (
            oc.reshape(R, B, F).transpose(1, 0, 2)
        )

    # Exact host-side affine epilogue (gamma/beta are data, not compile-time).
    gamma = np.asarray(inputs["ln_gamma"], np.float32)
    beta = np.asarray(inputs["ln_beta"], np.float32)
    if not (np.all(gamma == 1.0) and np.all(beta == 0.0)):
        out_full = out_full * gamma + beta
    return out_full, res


def kernel(**inputs) -> np.ndarray:
    out, _ = _run(inputs, trace=False)
    return out
